# revision 31
# baseline (speedup 1.0000x reference)
"""Trainium2 Bass kernel for the LTC (liquid time-constant) memory cell.

Model (see reference): v-state recurrence over T=128 timesteps, each with 6
ODE unfold iterations:
    v' = (cm_t*v + gl*vl + num_syn) / (cm_t + gl + den_syn + eps)
with 2 recurrent synapses per neuron (self: u, pair: (u+dim)%U) and one
sensory synapse (source d = u%dim).

Sharding: 8 cores; core c owns the 128 neuron *pairs* {u=c*128+p, u+1024}
for p in [0,128), with the FULL batch B=32. Each partition p holds one
pair; per-half state tiles [128,32] carried as w = v + 1 so the erev=-1
numerator collapses to q = cmt*w + nd (no recurrent term).

The run is one 768-round serial dependency ring (sigmoid -> den -> divide
-> sigmoid); throughput is irrelevant, only ring latency matters:
 - ALL sensory work (sensory sigmoids, ds/nd tiles for every timestep) is
   precomputed in a prologue with a few huge instructions, so the ring
   engines are never perturbed,
 - the state is carried PRE-SCALED per half (zA = sig0A*wA etc.), with a
   third scaled copy (zB2 = sig1A*wB) produced by an extra divide, so 3 of
   the 4 ring sigmoids use IMMEDIATE scale=1 (an AP scale operand costs
   ~90ns extra per ACT instruction and ACT does not pipeline dependents);
   all rescaling folds into host-precomputed constants,
 - den chains normalized by softplus(w1) so the outer combine is a plain
   add; inner combines are affine custom ops carrying w0/w1,
 - w' = q/den is a fused approximate-divide custom DVE op per scaled
   state (BITWISE_NOT bit-trick reciprocal with a quadratic Chebyshev
   seed, ~8e-5 rel err, 8 ALU stages, all consts literal),
 - q = cmt*z + nd runs on Pool (tensor_tensor with broadcast cmt tiles
   plus a ratio copy for the extra scaled state), off the DVE queue,
 - instruction emission order is tuned so each den's last-arriving
   sigmoid feeds it directly and the divides complete in the order the
   next round's sigmoid queue consumes them.
Input x is preloaded to SBUF once; the input affine folds into the
sensory ACT scale/bias on the host; output affine + state unscaling
applied on the host.
"""

import numpy as np

import concourse.bacc as bacc
import concourse.mybir as mybir
from concourse import tile
from concourse.bass_utils import run_bass_kernel_spmd

ODE_UNFOLDS = 6
EPS = 1e-8
B = 32
T = 128
DIM = 1024
U = 2 * DIM
NCORES = 8
P = 128  # partitions = pairs per core

F32 = mybir.dt.float32
AF = mybir.ActivationFunctionType
OP = mybir.AluOpType

# ---------------------------------------------------------------------------
# Custom DVE op: fused approximate divide  out = in1 / in0
# n = bitcast(~x); m = x*n lands in [-4.5,-4] for all finite x>0; a quadratic
# Chebyshev fit of 1/m then out = n*poly(m)*in1. ~8.3e-5 max rel err.
DIV_CONSTS = {"s0": -0.7066511871005156, "s1": -0.16633655560380672,
              "imm2": -0.013042133349701725}


def _ref_div(in0, in1, c0, c1, c2):
    x = in0.astype(np.float32)
    n = (~x.view(np.int32)).view(np.float32)
    m = (x * n).astype(np.float32)
    t = (np.float32(c2) * m).astype(np.float32)
    t = (t + np.float32(c1)).astype(np.float32)
    t = (t * m).astype(np.float32)
    t = (t + np.float32(c0)).astype(np.float32)
    r = (t * n).astype(np.float32)
    return (r * in1).astype(np.float32)


def _make_div_op():
    import concourse.dve_ops as dve_ops
    from concourse.dve_spec import (Spec, Src0, Src1, Bin, AluOp, lower,
                                    _has_src1, C0, C1, C2)
    from concourse.dve_uop import DveOpSpec

    name = "TENSOR_DIV_APPROX_ANT"
    for o in dve_ops.OPS:
        if o.name == name:
            return o
    _n = Bin(AluOp.BITWISE_NOT, Src0, Src0)
    _m = Src0 * _n
    body = ((C2 * _m + C1) * _m + C0) * _n * Src1
    spec = Spec(body=body, reference=_ref_div)
    row = max(dve_ops._SUB_OPCODE_FOR_NAME.values()) + 1
    assert row < 0x20
    dve_ops._SUB_OPCODE_FOR_NAME[name] = row
    shas = {}
    for ver in ("v3", "v4"):
        compiled = DveOpSpec(name=name, opcode=row, uops=lower(spec, ver=ver),
                             rd1_en=_has_src1(spec))
        shas[ver] = compiled.sha(ver)
    op = dve_ops.DveOp(name, spec, subdim=False, uops_sha=shas)
    dve_ops.OPS.append(op)
    dve_ops.CUSTOM_DVE_SPECS[name] = spec
    return op


# pp column indices (per half; half B adds NPARAM)
(C_SIG0, C_B0P, C_SIG1, C_B1P, C_W0, C_W1, C_SSIG, C_NSMS,
 C_SPSW, C_GCME, C_WPS, C_GGP) = range(12)
NPARAM = 12

PCHUNK = 2048  # prologue instruction width
PSUM_STATE = False
IMM_SIG = True


def _softplus(x):
    x = x.astype(np.float64)
    return np.log1p(np.exp(-np.abs(x))) + np.maximum(x, 0.0)


def _build_v4(wbufs=6):
    div_op = _make_div_op()
    nc = bacc.Bacc(trn_type="TRN2")
    xin_d = nc.dram_tensor("xin", [P, T * B], F32, kind="ExternalInput")
    pp_d = nc.dram_tensor("pp", [P, 2 * NPARAM], F32, kind="ExternalInput")
    pc_d = nc.dram_tensor("pc", [P, 128], F32, kind="ExternalInput")
    out_d = nc.dram_tensor("out", [P, B], F32, kind="ExternalOutput")
    TB = T * B

    with tile.TileContext(nc) as tc:
        with tc.tile_pool(name="const", bufs=1) as cpool, \
             tc.tile_pool(name="work", bufs=wbufs) as wpool, \
             tc.tile_pool(name="ps", bufs=1, space="PSUM") as pspool:
            xin = cpool.tile([P, TB], F32, tag="xin", name="xin_t")
            pp = cpool.tile([P, 2 * NPARAM], F32, tag="pp", name="pp_t")
            pc = cpool.tile([P, 128], F32, tag="pc", name="pc_t")
            nc.sync.dma_start(xin[:], xin_d[:])
            nc.sync.dma_start(pp[:], pp_d[:])
            nc.sync.dma_start(pc[:], pc_d[:])

            def par(h, c):
                j = h * NPARAM + c
                return pp[:, j:j + 1]

            cmtb = [pc[:, 0:32], pc[:, 32:64]]  # cmt broadcast per half
            ratb = [pc[:, 64:96], pc[:, 96:128]]  # sig ratio per half

            # ---- prologue: all sensory tiles for every timestep ----
            sga = [cpool.tile([P, TB], F32, tag=f"sga{h}", name=f"sga{h}")
                   for h in range(2)]
            dsa = [cpool.tile([P, TB], F32, tag=f"dsa{h}", name=f"dsa{h}")
                   for h in range(2)]
            nda = [cpool.tile([P, TB], F32, tag=f"nda{h}", name=f"nda{h}")
                   for h in range(2)]
            for h in range(2):
                for j in range(0, TB, PCHUNK):
                    sl = slice(j, j + PCHUNK)
                    nc.scalar.activation(sga[h][:, sl], xin[:, sl],
                                         AF.Sigmoid, bias=par(h, C_NSMS),
                                         scale=par(h, C_SSIG))
                    nc.vector.tensor_scalar(
                        dsa[h][:, sl], sga[h][:, sl], par(h, C_SPSW),
                        par(h, C_GCME), OP.mult, OP.add)
                    nc.vector.tensor_scalar(
                        nda[h][:, sl], sga[h][:, sl], par(h, C_WPS),
                        par(h, C_GGP), OP.mult, OP.add)

            # ---- ring state: PSUM tiles, in-place (ring order makes all
            # readers of round k finish before round k's writes) ----
            spool = pspool if PSUM_STATE else cpool
            wA = spool.tile([P, 32], F32, tag="wA", name="wA")
            wB = spool.tile([P, 32], F32, tag="wB", name="wB")
            sg4 = [spool.tile([P, 32], F32, tag=f"sg_t{j}", name=f"sg_t{j}")
                   for j in range(4)]
            wA2 = spool.tile([P, 32], F32, tag="wA2", name="wA2")
            wB2 = spool.tile([P, 32], F32, tag="wB2", name="wB2")
            ones = cpool.tile([P, 32], F32, tag="ones", name="ones")
            nc.vector.memset(ones[:], 1.0)
            # four scaled states so every ring sigmoid has immediate
            # scale=1: zA = sig0A*wA, zB = sig0B*wB, zA2 = sig1B*wA,
            # zB2 = sig1A*wB
            nc.scalar.activation(wA[:], ones[:], AF.Copy,
                                 scale=par(0, C_SIG0))
            nc.scalar.activation(wB[:], ones[:], AF.Copy,
                                 scale=par(1, C_SIG0))
            nc.scalar.activation(wA2[:], ones[:], AF.Copy,
                                 scale=par(1, C_SIG1))
            nc.scalar.activation(wB2[:], ones[:], AF.Copy,
                                 scale=par(0, C_SIG1))

            def wtile(tag, n=32):
                return wpool.tile([P, n], F32, tag=tag, name=tag)

            for t in range(T):
                bs = slice(t * B, (t + 1) * B)
                for k in range(ODE_UNFOLDS):
                    s0A, s1B, s1A, s0B = (s[:] for s in sg4)
                    # ring sigmoids. State is carried pre-scaled
                    # (zA = sig1B*wA, zB = sig1A*wB) so the den-chain outer
                    # sigmoids (s1B reads zA, s1A reads zB) use IMMEDIATE
                    # scale=1 (saves ~90ns ACT issue each); the inner
                    # sigmoids use the per-partition ratio scale.
                    nc.scalar.activation(s0A, wA[:], AF.Sigmoid,
                                         bias=par(0, C_B0P),
                                         scale=1.0)
                    nc.scalar.activation(s0B, wB[:], AF.Sigmoid,
                                         bias=par(1, C_B0P),
                                         scale=1.0)
                    nc.scalar.activation(s1A, wB2[:], AF.Sigmoid,
                                         bias=par(0, C_B1P),
                                         scale=1.0)
                    nc.scalar.activation(s1B, wA[:], AF.Sigmoid,
                                         bias=par(1, C_B1P),
                                         scale=par(1, C_W1))
                    # q = cmt*w + nd on Pool, off the DVE queue
                    qA = wtile("qA")
                    qB = wtile("qB")
                    qmA = wtile("qmA")
                    qmB = wtile("qmB")
                    qB2 = wtile("qB2")
                    nc.gpsimd.tensor_tensor(qmA[:], wA[:], cmtb[0], OP.mult)
                    nc.gpsimd.tensor_tensor(qA[:], qmA[:], nda[0][:, bs],
                                            OP.add)
                    nc.gpsimd.tensor_tensor(qmB[:], wB[:], cmtb[1], OP.mult)
                    nc.gpsimd.tensor_tensor(qB[:], qmB[:], nda[1][:, bs],
                                            OP.add)
                    nc.gpsimd.tensor_tensor(qB2[:], qB[:], ratb[1], OP.mult)
                    # den chains on DVE
                    d1A = wtile("d1A")
                    d1B = wtile("d1B")
                    denA = wtile("denA")
                    denB = wtile("denB")
                    # affine_then_add (custom DVE) instead of native STT:
                    # custom ops issue faster (~192 vs 251ns) and an
                    # all-custom DVE stream avoids the native<->custom
                    # mode-switch stall before the divides
                    nc.vector.affine_then_add(
                        d1A[:], s0A, dsa[0][:, bs], par(0, C_W0), 0.0)
                    nc.vector.affine_then_add(
                        d1B[:], s0B, dsa[1][:, bs], par(1, C_W0), 0.0)
                    # custom-op adds: keeps the DVE stream all-custom
                    # (native<->custom switches cost ~157ns)
                    nc.vector.affine_then_add(denA[:], s1A, d1A[:], 1.0, 0.0)
                    nc.vector.affine_then_add(denB[:], s1B, d1B[:], 1.0, 0.0)
                    # four divides produce the four scaled states, in the
                    # order the next round's sigmoid queue consumes them
                    nc.vector._custom_dve(
                        div_op, out=wA[:], in0=denA[:], in1=qA[:],
                        s0=DIV_CONSTS["s0"], s1=DIV_CONSTS["s1"],
                        imm2=DIV_CONSTS["imm2"])
                    nc.vector._custom_dve(
                        div_op, out=wB[:], in0=denB[:], in1=qB[:],
                        s0=DIV_CONSTS["s0"], s1=DIV_CONSTS["s1"],
                        imm2=DIV_CONSTS["imm2"])
                    nc.vector._custom_dve(
                        div_op, out=wB2[:], in0=denB[:], in1=qB2[:],
                        s0=DIV_CONSTS["s0"], s1=DIV_CONSTS["s1"],
                        imm2=DIV_CONSTS["imm2"])


            outt = cpool.tile([P, B], F32, tag="outt", name="outt")
            nc.vector.tensor_copy(outt[:], wA[:])
            nc.sync.dma_start(out_d[:], outt[:])
    nc.compile()
    return nc


_NC_CACHE = {}


def _get_nc():
    if "v4" not in _NC_CACHE:
        _NC_CACHE["v4"] = _build_v4()
    return _NC_CACHE["v4"]


def _host_params(c, gleak, vleak, cm, w, sigma, mu, erev,
                 sens_w, sens_sigma, sens_mu, sens_erev,
                 input_w, input_b):
    """pp [128, 2*NPARAM] and pc [128, 64] for core c."""
    d = c * P + np.arange(P)
    pp = np.zeros((P, 2 * NPARAM), np.float32)
    pcn = np.zeros((P, 4, 32), np.float32)
    for h in range(2):
        u = h * DIM + d
        sp_w = _softplus(w[u])
        sp_gl = _softplus(gleak[u])
        sp_sw = _softplus(sens_w[u])
        cmt = _softplus(cm[u]) * ODE_UNFOLDS
        o = h * NPARAM
        uo = (1 - h) * DIM + d  # partner half
        # state carried scaled: z_h = sigma[u,0] * w_h (the own-half inner
        # sigmoid reads this state with immediate scale=1); the partner's
        # slot-1 sigmoid uses the AP ratio below
        zscl = sigma[u, 0]
        # state shift w = v + 1: sigmoid biases absorb -sigma
        pp[:, o + C_SIG0] = sigma[u, 0]  # raw: state init (z = sig0*w)
        pp[:, o + C_B0P] = -(mu[u, 0] + 1.0) * sigma[u, 0]
        pp[:, o + C_SIG1] = sigma[u, 1]  # raw: z2 init (z2 = sig1*w)
        pp[:, o + C_B1P] = -(mu[u, 1] + 1.0) * sigma[u, 1]
        pp[:, o + C_W0] = sp_w[:, 0]
        pp[:, o + C_W1] = sp_w[:, 1]
        pp[:, o + C_SSIG] = sens_sigma[u] * input_w[d]
        pp[:, o + C_NSMS] = (input_b[d] - sens_mu[u]) * sens_sigma[u]
        pp[:, o + C_SPSW] = sp_sw
        pp[:, o + C_GCME] = cmt + sp_gl + EPS
        pp[:, o + C_WPS] = sp_sw * (1.0 + sens_erev[u]) * zscl
        pp[:, o + C_GGP] = (sp_gl * vleak[u] + sp_gl + EPS) * zscl
        pcn[:, h, :] = cmt[:, None]
        # den chains normalized by softplus(w1) so the outer combine is a
        # plain tensor add: scale the d1/ds/q constants accordingly
        w1 = sp_w[:, 1]
        pp[:, o + C_W0] /= w1
        # C_W1 repurposed: AP ratio scale for the s1 sigmoid reading the
        # partner's z state (z = sig0*w)
        pp[:, o + C_W1] = sigma[u, 1] / sigma[uo, 0]
        pp[:, o + C_SPSW] /= w1
        pp[:, o + C_GCME] /= w1
        pp[:, o + C_WPS] /= w1
        pp[:, o + C_GGP] /= w1
        pcn[:, h, :] = (cmt / w1)[:, None]
        # ratio for the second scaled state: z2_h = sig1[partner] * w_h,
        # built on Pool as q2 = (sig1[partner]/sig0[own]) * q
        pcn[:, 2 + h, :] = (sigma[uo, 1] / sigma[u, 0])[:, None]
    return pp, pcn.reshape(P, 128).astype(np.float32)


def kernel(inputs, gleak, vleak, cm, w, sigma, mu, erev,
           sens_w, sens_sigma, sens_mu, sens_erev,
           input_w, input_b, output_w, output_b, _trace=False):
    inputs = np.asarray(inputs, np.float32)
    args = dict(gleak=np.asarray(gleak, np.float32),
                vleak=np.asarray(vleak, np.float32),
                cm=np.asarray(cm, np.float32),
                w=np.asarray(w, np.float32),
                sigma=np.asarray(sigma, np.float32),
                mu=np.asarray(mu, np.float32),
                erev=np.asarray(erev, np.float32),
                sens_w=np.asarray(sens_w, np.float32),
                sens_sigma=np.asarray(sens_sigma, np.float32),
                sens_mu=np.asarray(sens_mu, np.float32),
                sens_erev=np.asarray(sens_erev, np.float32),
                input_w=np.asarray(input_w, np.float32),
                input_b=np.asarray(input_b, np.float32))
    # the fused numerator (q = cmt*w + nd) relies on erev == -1 exactly
    assert np.allclose(args["erev"], -1.0), "kernel requires erev == -1"

    in_maps = []
    for c in range(NCORES):
        xc = inputs[:, :, c * P:(c + 1) * P]          # [B,T,P]
        xin = np.ascontiguousarray(
            xc.transpose(2, 1, 0).reshape(P, T * B))  # [P, t*B+b]
        pp, pcn = _host_params(c, **args)
        in_maps.append({"xin": xin, "pp": pp, "pc": pcn})

    nc = _get_nc()
    res = run_bass_kernel_spmd(nc, in_maps, core_ids=list(range(NCORES)),
                               trace=_trace)

    out = np.zeros((B, DIM), np.float32)
    for c in range(NCORES):
        out[:, c * P:(c + 1) * P] = res.results[c]["out"].T
    out = out / args["sigma"][:DIM, 0]  # z = sig0A * w
    out = out - 1.0  # state was carried as w = v + 1
    out = out * np.asarray(output_w, np.float32) + np.asarray(output_b, np.float32)
    if _trace:
        kernel.last_results = res
    return out
